# revision 8
# baseline (speedup 1.0000x reference)
"""GCNConv (PyG, bias=False) on 8 Trainium2 NeuronCores.

out = D^{-1/2} (A+I) D^{-1/2} (x @ W)

Strategy: the op is linear, so aggregate first, project second:
  z = dis * x                     (host; dis = rsqrt(degree), z stored bf16)
  aggT[f,d] = sum_{src->d} z[src] (device: dma_gather + one-hot matmul)
  out[d] = dis[d] * (agg[d] @ W)  (device: dis applied at the PSUM flush)

Node rows (outputs) are partitioned across the 8 cores; each core's edges are
sorted by (128-dst window, 25000-row src chunk, src).  Per (window, chunk)
group, dma_gather pulls z rows by local int16 source index in calls of up to
1024 rows; trailing padding uses idx -1 (descriptors skipped — per-call true
counts are value_load-ed from an input tensor so the shared SPMD NEFF works
for every core's edge counts).  One batched DVE tensor_tensor builds the
one-hot blocks S[e, t, d] = (dstoff[e,t] == d) for a whole call; the PE
accumulates aggT += slab_tile^T @ S_tile in PSUM over the window, then
aggT feeds matmul(lhsT=aggT, rhs=W) directly (contraction dim = feat is
already on partitions) and the flush multiplies by dis[dst] per partition.
No transposes are needed anywhere.  Gather slabs rotate through 6 fixed
slots, memset once, so rows skipped by short gathers always hold finite
stale data that the zero one-hot rows annihilate.
"""
import os
import sys

sys.path.insert(0, '/opt/trn_rl_repo')

import numpy as np

N_NODES = 100000
N_EDGES = 1600000
DIM = 128
N_CORES = 8
NPC = N_NODES // N_CORES          # dst rows per core (12500)
WIN = 128                         # dsts per window
NW = (NPC + WIN - 1) // WIN       # windows per core (98; last window 84 dsts)
CHUNK = 25000                     # src rows per gather-table chunk (int16 limit)
NQ = (N_NODES + CHUNK - 1) // CHUNK
TILE = 128                        # edges per tile
MAX_CALL_TILES = 8                # 1024 idxs per dma_gather (64-desc/engine cap)
N_SLABS = 6

_patched = False


def _setup_concourse():
    global _patched
    if _patched:
        return
    _patched = True
    import bass_rust
    import concourse.bass as bass
    import concourse.tile as tile

    # Walrus in this container allows exactly ONE sync-wait per instruction.
    # (1) Tile's end-of-context drain can carry several: split extra waits
    # onto chained Drain instructions.
    def _patched_drain_and_barrier(self, tick_clock, wait_clock):
        from concourse.vector_clock import ScopedClock
        nc = self.nc
        drain_inst = nc.sync.drain()
        wait_clock.add_sem_waits(drain_inst.ins,
                                 ScopedClock({None: tick_clock.global_clock}))
        si = drain_inst.ins.sync_info
        waits = list(si.on_wait or []) if si is not None else []
        if len(waits) > 1:
            si.on_wait = waits[:1]
            for w in waits[1:]:
                d2 = nc.sync.drain()
                d2.ins.sync_info = bass_rust.SyncInfo(on_wait=[w], on_update=[])
        nc.all_engine_barrier()
        popped = nc._tile_sem_poison_stack.pop()
        assert popped is self._sem_poison
        nc.clear_and_free_semaphores(list(self.sems.allocated().values()))
        nc.all_engine_barrier()

    tile.TileContext._drain_and_barrier = _patched_drain_and_barrier

    # (2) Any other instruction with >1 waits: move extras onto NoOp
    # carriers on the same engine immediately before it.
    def _legalize_waits(m):
        for f in m.functions:
            for blk in f.blocks:
                insts = blk.instructions
                out = []
                changed = False
                for inst in insts:
                    si = inst.sync_info
                    waits = list(si.on_wait or []) if si is not None else []
                    if len(waits) > 1:
                        changed = True
                        for k, w in enumerate(waits[:-1]):
                            nop = bass_rust.InstNoOp(
                                name=f"{inst.name}-wsplit{k}", ins=[], outs=[])
                            nop.engine = inst.engine
                            nop.sync_info = bass_rust.SyncInfo(
                                on_wait=[w], on_update=[])
                            out.append(nop)
                        si.on_wait = waits[-1:]
                    out.append(inst)
                if changed:
                    blk.instructions = out

    orig_to_json_bytes = bass.Bass.to_json_bytes
    if not getattr(bass.Bass, "_wsplit_patch", False):
        def _patched_to_json_bytes(self):
            _legalize_waits(self.m)
            return orig_to_json_bytes(self)
        bass.Bass.to_json_bytes = _patched_to_json_bytes
        bass.Bass._wsplit_patch = True


def _plan_calls(T):
    """Split each (w, q) group's tiles into dma_gather calls of <=8 tiles.
    Returns [(w, q, tile_off_in_group, n_tiles, global_tile_idx)] in order."""
    calls = []
    gt = 0
    for w in range(NW):
        for q in range(NQ):
            tq = int(T[w, q])
            c0 = 0
            while c0 < tq:
                nt = min(MAX_CALL_TILES, tq - c0)
                calls.append((w, q, c0, nt, gt))
                gt += nt
                c0 += nt
    return calls


def _preprocess(x, edge_index, W):
    """Host-side sharding: per-core padded edge arrays + shared schedule."""
    import ml_dtypes
    x = np.asarray(x, dtype=np.float32)
    W = np.asarray(W, dtype=np.float32)
    ei = np.asarray(edge_index)
    src = np.concatenate([ei[0], np.arange(N_NODES, dtype=ei.dtype)]).astype(np.int64)
    dst = np.concatenate([ei[1], np.arange(N_NODES, dtype=ei.dtype)]).astype(np.int64)

    deg = np.bincount(dst, minlength=N_NODES).astype(np.float32)
    dis = 1.0 / np.sqrt(np.maximum(deg, 1.0))
    z = (x * dis[:, None]).astype(ml_dtypes.bfloat16)    # gather table (bf16)

    core = dst // NPC
    dloc = dst - core * NPC
    w = dloc // WIN
    dstoff = (dloc - w * WIN).astype(np.float32)
    q = src // CHUNK
    srcloc = (src - q * CHUNK).astype(np.int16)

    key = (core * NW + w) * NQ + q
    order = np.lexsort((src, key))                       # by group, then src
    key_s = key[order]
    cnt = np.bincount(key, minlength=N_CORES * NW * NQ).reshape(N_CORES, NW, NQ)
    T = (-(-cnt // TILE)).max(axis=0)                    # [NW, NQ] tiles/group
    group_off = np.concatenate([[0], np.cumsum(T.reshape(-1) * TILE)])
    L = int(group_off[-1])                               # padded edges per core

    first_idx = np.searchsorted(key_s, np.arange(N_CORES * NW * NQ), side='left')
    rank = np.arange(key_s.size) - first_idx[key_s]
    pos = group_off[key_s % (NW * NQ)] + rank

    calls = _plan_calls(T)
    ncalls = len(calls)

    srcloc_s = srcloc[order]
    dstoff_s = dstoff[order]
    core_s = key_s // (NW * NQ)
    idx_arrs, dst_arrs, cnt_arrs, disw_arrs = [], [], [], []
    for c in range(N_CORES):
        m = core_s == c
        ia = np.full(L, -1, np.int16)       # pad: skipped by true count
        da = np.full(L, -1.0, np.float32)   # pad: matches no iota column
        p = pos[m]
        ia[p] = srcloc_s[m]
        da[p] = dstoff_s[m]
        # per-call true index counts (>=1; empty calls gather row 0 once)
        counts = np.zeros(ncalls, np.int32)
        for ci, (wq, qq, c0, nt, gt) in enumerate(calls):
            g = wq * NQ + qq
            real = int(cnt[c, wq, qq]) - c0 * TILE
            real = max(0, min(real, nt * TILE))
            if real == 0:
                ia[gt * TILE] = 0
                real = 1
            counts[ci] = real
        cnt_arrs.append(np.ascontiguousarray(
            np.tile(counts[None, :], (128, 1))))
        idx_arrs.append(np.ascontiguousarray(
            np.tile(ia.reshape(-1, 16).T, (8, 1))))
        dst_arrs.append(np.ascontiguousarray(
            da.reshape(-1, TILE).T.astype(ml_dtypes.bfloat16)))
        # dis of this core's dst rows, [128, NW] (partition p, window w)
        dw = np.zeros((128, NW), np.float32)
        rows = np.arange(NPC)
        dw[rows % WIN, rows // WIN] = dis[c * NPC + rows]
        disw_arrs.append(np.ascontiguousarray(dw))

    iota = np.ascontiguousarray(
        np.tile(np.arange(WIN, dtype=np.float32), (TILE, 1))
    ).astype(ml_dtypes.bfloat16)
    return z, W, T, calls, idx_arrs, dst_arrs, cnt_arrs, disw_arrs, iota


def _build(T, calls):
    """Build the shared SPMD bass program from the schedule."""
    import concourse.bacc as bacc
    import concourse.mybir as mybir
    import concourse.tile as tile

    tot_tiles = int(T.sum())
    L = tot_tiles * TILE
    ncalls = len(calls)
    bf16 = mybir.dt.bfloat16
    f32 = mybir.dt.float32

    nc = bacc.Bacc("TRN2", target_bir_lowering=False, debug=False)
    z_ds = [nc.dram_tensor(f"z{q}", [min(CHUNK, N_NODES - q * CHUNK), DIM],
                           bf16, kind="ExternalInput")
            for q in range(NQ)]
    idx_d = nc.dram_tensor("idxs", [128, L // 16], mybir.dt.int16, kind="ExternalInput")
    dst_d = nc.dram_tensor("dstv", [128, tot_tiles], bf16, kind="ExternalInput")
    cnt_d = nc.dram_tensor("cnts", [128, ncalls], mybir.dt.int32, kind="ExternalInput")
    disw_d = nc.dram_tensor("disw", [128, NW], f32, kind="ExternalInput")
    iota_d = nc.dram_tensor("iota", [128, WIN], bf16, kind="ExternalInput")
    W_d = nc.dram_tensor("W", [DIM, DIM], f32, kind="ExternalInput")
    out_d = nc.dram_tensor("out", [NPC, DIM], f32, kind="ExternalOutput")

    with tile.TileContext(nc) as tc:
        with tc.tile_pool(name="const", bufs=1) as cpool, \
             tc.tile_pool(name="slabs", bufs=1) as slpool, \
             tc.tile_pool(name="sel", bufs=4) as spool, \
             tc.tile_pool(name="stage", bufs=3) as apool, \
             tc.tile_pool(name="pagg", bufs=3, space="PSUM") as pagg, \
             tc.tile_pool(name="pout", bufs=2, space="PSUM") as pout:

            idxs = cpool.tile([128, L // 16], mybir.dt.int16)
            nc.sync.dma_start(out=idxs[:], in_=idx_d[:])
            dstv = cpool.tile([128, tot_tiles], bf16)
            nc.sync.dma_start(out=dstv[:], in_=dst_d[:])
            cnts = cpool.tile([128, ncalls], mybir.dt.int32)
            nc.sync.dma_start(out=cnts[:], in_=cnt_d[:])
            disw = cpool.tile([128, NW], f32)
            nc.sync.dma_start(out=disw[:], in_=disw_d[:])
            iota = cpool.tile([128, WIN], bf16)
            nc.sync.dma_start(out=iota[:], in_=iota_d[:])
            Wt = cpool.tile([DIM, DIM], f32)
            nc.sync.dma_start(out=Wt[:], in_=W_d[:])

            # fixed gather slots, memset once -> unwritten rows stay finite
            slabs = []
            for i in range(N_SLABS):
                s = slpool.tile([128, MAX_CALL_TILES, DIM], bf16, tag=f"slab{i}")
                nc.vector.memset(s[:], 0.0)
                slabs.append(s)

            creg = nc.gpsimd.alloc_register("gather_cnt")

            # group calls by window for PSUM accumulation bookkeeping
            w_first = {}
            w_last = {}
            for ci, (w, q, c0, nt, gt) in enumerate(calls):
                w_first.setdefault(w, ci)
                w_last[w] = ci

            psum_agg = None
            ti_in_w = 0
            tiles_w = 0
            for ci, (w, q, c0, nt, gt) in enumerate(calls):
                if w_first[w] == ci:
                    psum_agg = pagg.tile([128, WIN], f32, tag="pagg")
                    ti_in_w = 0
                    tiles_w = int(T[w].sum())
                slab = slpool.tile([128, MAX_CALL_TILES, DIM], bf16,
                                  tag=f"slab{ci % N_SLABS}")
                n_idx = nt * TILE
                nc.gpsimd.reg_load(creg, cnts[0:1, ci:ci + 1])
                nc.gpsimd.dma_gather(
                    slab[:, :nt, :], z_ds[q][:],
                    idxs[:, (gt * TILE) // 16:(gt * TILE + n_idx) // 16],
                    n_idx, creg, DIM)
                # one batched one-hot build for the whole call
                S = spool.tile([TILE, MAX_CALL_TILES, WIN], bf16, tag="S")
                nc.vector.tensor_tensor(
                    out=S[:, :nt, :],
                    in0=iota[:].rearrange("p (t j) -> p t j", t=1)
                        .to_broadcast([TILE, nt, WIN]),
                    in1=dstv[:, gt:gt + nt]
                        .rearrange("p (t j) -> p t j", j=1)
                        .to_broadcast([TILE, nt, WIN]),
                    op=mybir.AluOpType.is_equal)
                for t in range(nt):
                    nc.tensor.matmul(
                        out=psum_agg[:], lhsT=slab[:, t, :], rhs=S[:, t, :],
                        start=(ti_in_w == 0), stop=(ti_in_w == tiles_w - 1))
                    ti_in_w += 1
                if w_last[w] == ci:
                    wlen = min(WIN, NPC - w * WIN)
                    aggT = apool.tile([128, WIN], f32, tag="aggT")
                    nc.vector.tensor_copy(out=aggT[:], in_=psum_agg[:])
                    psum_o = pout.tile([WIN, DIM], f32)
                    nc.tensor.matmul(out=psum_o[:], lhsT=aggT[:], rhs=Wt[:],
                                     start=True, stop=True)
                    osb = apool.tile([WIN, DIM], f32, tag="osb")
                    nc.vector.tensor_scalar(
                        out=osb[:], in0=psum_o[:],
                        scalar1=disw[:, w:w + 1], scalar2=None,
                        op0=mybir.AluOpType.mult)
                    nc.sync.dma_start(out=out_d[w * WIN:w * WIN + wlen, :],
                                      in_=osb[:wlen, :])
    nc.compile()
    return nc


def kernel(x, edge_index, W):
    _setup_concourse()
    from concourse.bass_utils import run_bass_kernel_spmd

    z, W32, T, calls, idx_arrs, dst_arrs, cnt_arrs, disw_arrs, iota = \
        _preprocess(x, edge_index, W)
    nc = _build(T, calls)

    in_maps = []
    for c in range(N_CORES):
        im = {"idxs": idx_arrs[c], "dstv": dst_arrs[c], "cnts": cnt_arrs[c],
              "disw": disw_arrs[c], "iota": iota, "W": W32}
        for q in range(NQ):
            im[f"z{q}"] = np.ascontiguousarray(z[q * CHUNK:(q + 1) * CHUNK])
        in_maps.append(im)
    res = run_bass_kernel_spmd(nc, in_maps, core_ids=list(range(N_CORES)))
    out = np.empty((N_NODES, DIM), np.float32)
    for c in range(N_CORES):
        out[c * NPC:(c + 1) * NPC] = res.results[c]["out"]
    return out


# revision 9
# speedup vs baseline: 1.0006x; 1.0006x over previous
"""GCNConv (PyG, bias=False) on 8 Trainium2 NeuronCores.

out = D^{-1/2} (A+I) D^{-1/2} (x @ W)

Strategy: the op is linear, so aggregate first, project second:
  z = dis * x                     (host; dis = rsqrt(degree), z stored bf16)
  aggT[f,d] = sum_{src->d} z[src] (device: dma_gather + one-hot matmul)
  out[d] = dis[d] * (agg[d] @ W)  (device: dis applied at the PSUM flush)

Node rows (outputs) are partitioned across the 8 cores; each core's edges are
sorted by (128-dst window, 25000-row src chunk, src).  Per (window, chunk)
group, dma_gather pulls z rows by local int16 source index in calls of up to
1024 rows; trailing padding uses idx -1 (descriptors skipped — per-call true
counts are value_load-ed from an input tensor so the shared SPMD NEFF works
for every core's edge counts).  One batched DVE tensor_tensor builds the
one-hot blocks S[e, t, d] = (dstoff[e,t] == d) for a whole call; the PE
accumulates aggT += slab_tile^T @ S_tile in PSUM over the window, then
aggT feeds matmul(lhsT=aggT, rhs=W) directly (contraction dim = feat is
already on partitions) and the flush multiplies by dis[dst] per partition.
No transposes are needed anywhere.  Gather slabs rotate through 6 fixed
slots, memset once, so rows skipped by short gathers always hold finite
stale data that the zero one-hot rows annihilate.
"""
import os
import sys

sys.path.insert(0, '/opt/trn_rl_repo')

import numpy as np

N_NODES = 100000
N_EDGES = 1600000
DIM = 128
N_CORES = 8
NPC = N_NODES // N_CORES          # dst rows per core (12500)
WIN = 128                         # dsts per window
NW = (NPC + WIN - 1) // WIN       # windows per core (98; last window 84 dsts)
CHUNK = 25000                     # src rows per gather-table chunk (int16 limit)
NQ = (N_NODES + CHUNK - 1) // CHUNK
TILE = 128                        # edges per tile
MAX_CALL_TILES = 8                # 1024 idxs per dma_gather (64-desc/engine cap)
N_SLABS = 6

_patched = False


def _setup_concourse():
    global _patched
    if _patched:
        return
    _patched = True
    import bass_rust
    import concourse.bass as bass
    import concourse.tile as tile

    # Walrus in this container allows exactly ONE sync-wait per instruction.
    # (1) Tile's end-of-context drain can carry several: split extra waits
    # onto chained Drain instructions.
    def _patched_drain_and_barrier(self, tick_clock, wait_clock):
        from concourse.vector_clock import ScopedClock
        nc = self.nc
        drain_inst = nc.sync.drain()
        wait_clock.add_sem_waits(drain_inst.ins,
                                 ScopedClock({None: tick_clock.global_clock}))
        si = drain_inst.ins.sync_info
        waits = list(si.on_wait or []) if si is not None else []
        if len(waits) > 1:
            si.on_wait = waits[:1]
            for w in waits[1:]:
                d2 = nc.sync.drain()
                d2.ins.sync_info = bass_rust.SyncInfo(on_wait=[w], on_update=[])
        nc.all_engine_barrier()
        popped = nc._tile_sem_poison_stack.pop()
        assert popped is self._sem_poison
        nc.clear_and_free_semaphores(list(self.sems.allocated().values()))
        nc.all_engine_barrier()

    tile.TileContext._drain_and_barrier = _patched_drain_and_barrier

    # (2) Any other instruction with >1 waits: move extras onto NoOp
    # carriers on the same engine immediately before it.
    def _legalize_waits(m):
        for f in m.functions:
            for blk in f.blocks:
                insts = blk.instructions
                out = []
                changed = False
                for inst in insts:
                    si = inst.sync_info
                    waits = list(si.on_wait or []) if si is not None else []
                    if len(waits) > 1:
                        changed = True
                        for k, w in enumerate(waits[:-1]):
                            nop = bass_rust.InstNoOp(
                                name=f"{inst.name}-wsplit{k}", ins=[], outs=[])
                            nop.engine = inst.engine
                            nop.sync_info = bass_rust.SyncInfo(
                                on_wait=[w], on_update=[])
                            out.append(nop)
                        si.on_wait = waits[-1:]
                    out.append(inst)
                if changed:
                    blk.instructions = out

    orig_to_json_bytes = bass.Bass.to_json_bytes
    if not getattr(bass.Bass, "_wsplit_patch", False):
        def _patched_to_json_bytes(self):
            _legalize_waits(self.m)
            return orig_to_json_bytes(self)
        bass.Bass.to_json_bytes = _patched_to_json_bytes
        bass.Bass._wsplit_patch = True


def _plan_calls(T):
    """Split each (w, q) group's tiles into dma_gather calls of <=8 tiles.
    Returns [(w, q, tile_off_in_group, n_tiles, global_tile_idx)] in order."""
    calls = []
    gt = 0
    for w in range(NW):
        for q in range(NQ):
            tq = int(T[w, q])
            c0 = 0
            while c0 < tq:
                nt = min(MAX_CALL_TILES, tq - c0)
                calls.append((w, q, c0, nt, gt))
                gt += nt
                c0 += nt
    return calls


def _preprocess(x, edge_index, W):
    """Host-side sharding: per-core padded edge arrays + shared schedule."""
    import ml_dtypes
    x = np.asarray(x, dtype=np.float32)
    W = np.asarray(W, dtype=np.float32)
    ei = np.asarray(edge_index)
    src = np.concatenate([ei[0], np.arange(N_NODES, dtype=ei.dtype)]).astype(np.int64)
    dst = np.concatenate([ei[1], np.arange(N_NODES, dtype=ei.dtype)]).astype(np.int64)

    deg = np.bincount(dst, minlength=N_NODES).astype(np.float32)
    dis = 1.0 / np.sqrt(np.maximum(deg, 1.0))
    z = (x * dis[:, None]).astype(ml_dtypes.bfloat16)    # gather table (bf16)

    core = dst // NPC
    dloc = dst - core * NPC
    w = dloc // WIN
    dstoff = (dloc - w * WIN).astype(np.float32)
    q = src // CHUNK
    srcloc = (src - q * CHUNK).astype(np.int16)

    key = (core * NW + w) * NQ + q
    order = np.lexsort((src, key))                       # by group, then src
    key_s = key[order]
    cnt = np.bincount(key, minlength=N_CORES * NW * NQ).reshape(N_CORES, NW, NQ)
    T = (-(-cnt // TILE)).max(axis=0)                    # [NW, NQ] tiles/group
    group_off = np.concatenate([[0], np.cumsum(T.reshape(-1) * TILE)])
    L = int(group_off[-1])                               # padded edges per core

    first_idx = np.searchsorted(key_s, np.arange(N_CORES * NW * NQ), side='left')
    rank = np.arange(key_s.size) - first_idx[key_s]
    pos = group_off[key_s % (NW * NQ)] + rank

    calls = _plan_calls(T)
    ncalls = len(calls)

    srcloc_s = srcloc[order]
    dstoff_s = dstoff[order]
    core_s = key_s // (NW * NQ)
    idx_arrs, dst_arrs, cnt_arrs, disw_arrs = [], [], [], []
    for c in range(N_CORES):
        m = core_s == c
        ia = np.full(L, -1, np.int16)       # pad: skipped by true count
        da = np.full(L, -1.0, np.float32)   # pad: matches no iota column
        p = pos[m]
        ia[p] = srcloc_s[m]
        da[p] = dstoff_s[m]
        # per-call true index counts (>=1; empty calls gather row 0 once)
        counts = np.zeros(ncalls, np.int32)
        for ci, (wq, qq, c0, nt, gt) in enumerate(calls):
            g = wq * NQ + qq
            real = int(cnt[c, wq, qq]) - c0 * TILE
            real = max(0, min(real, nt * TILE))
            if real == 0:
                ia[gt * TILE] = 0
                real = 1
            counts[ci] = real
        cnt_arrs.append(np.ascontiguousarray(
            np.tile(counts[None, :], (128, 1))))
        idx_arrs.append(np.ascontiguousarray(
            np.tile(ia.reshape(-1, 16).T, (8, 1))))
        dst_arrs.append(np.ascontiguousarray(
            da.reshape(-1, TILE).T.astype(ml_dtypes.bfloat16)))
        # dis of this core's dst rows, [128, NW] (partition p, window w)
        dw = np.zeros((128, NW), np.float32)
        rows = np.arange(NPC)
        dw[rows % WIN, rows // WIN] = dis[c * NPC + rows]
        disw_arrs.append(np.ascontiguousarray(dw))

    iota = np.ascontiguousarray(
        np.tile(np.arange(WIN, dtype=np.float32), (TILE, 1))
    ).astype(ml_dtypes.bfloat16)
    return z, W, T, calls, idx_arrs, dst_arrs, cnt_arrs, disw_arrs, iota


def _build(T, calls):
    """Build the shared SPMD bass program from the schedule."""
    import concourse.bacc as bacc
    import concourse.mybir as mybir
    import concourse.tile as tile

    tot_tiles = int(T.sum())
    L = tot_tiles * TILE
    ncalls = len(calls)
    bf16 = mybir.dt.bfloat16
    f32 = mybir.dt.float32

    nc = bacc.Bacc("TRN2", target_bir_lowering=False, debug=False)
    z_ds = [nc.dram_tensor(f"z{q}", [min(CHUNK, N_NODES - q * CHUNK), DIM],
                           bf16, kind="ExternalInput")
            for q in range(NQ)]
    idx_d = nc.dram_tensor("idxs", [128, L // 16], mybir.dt.int16, kind="ExternalInput")
    dst_d = nc.dram_tensor("dstv", [128, tot_tiles], bf16, kind="ExternalInput")
    cnt_d = nc.dram_tensor("cnts", [128, ncalls], mybir.dt.int32, kind="ExternalInput")
    disw_d = nc.dram_tensor("disw", [128, NW], f32, kind="ExternalInput")
    iota_d = nc.dram_tensor("iota", [128, WIN], bf16, kind="ExternalInput")
    W_d = nc.dram_tensor("W", [DIM, DIM], f32, kind="ExternalInput")
    out_d = nc.dram_tensor("out", [NPC, DIM], f32, kind="ExternalOutput")

    with tile.TileContext(nc) as tc:
        with tc.tile_pool(name="const", bufs=1) as cpool, \
             tc.tile_pool(name="slabs", bufs=1) as slpool, \
             tc.tile_pool(name="sel", bufs=4) as spool, \
             tc.tile_pool(name="stage", bufs=3) as apool, \
             tc.tile_pool(name="pagg", bufs=3, space="PSUM") as pagg, \
             tc.tile_pool(name="pout", bufs=2, space="PSUM") as pout:

            idxs = cpool.tile([128, L // 16], mybir.dt.int16)
            nc.sync.dma_start(out=idxs[:], in_=idx_d[:])
            dstv = cpool.tile([128, tot_tiles], bf16)
            nc.sync.dma_start(out=dstv[:], in_=dst_d[:])
            cnts = cpool.tile([128, ncalls], mybir.dt.int32)
            nc.sync.dma_start(out=cnts[:], in_=cnt_d[:])
            disw = cpool.tile([128, NW], f32)
            nc.sync.dma_start(out=disw[:], in_=disw_d[:])
            iota = cpool.tile([128, WIN], bf16)
            nc.sync.dma_start(out=iota[:], in_=iota_d[:])
            Wt = cpool.tile([DIM, DIM], f32)
            nc.sync.dma_start(out=Wt[:], in_=W_d[:])

            # fixed gather slots, memset once -> unwritten rows stay finite
            slabs = []
            for i in range(N_SLABS):
                s = slpool.tile([128, MAX_CALL_TILES, DIM], bf16, tag=f"slab{i}")
                nc.vector.memset(s[:], 0.0)
                slabs.append(s)

            cregs = [nc.gpsimd.alloc_register(f"gather_cnt{i}")
                     for i in range(N_SLABS)]

            # group calls by window for PSUM accumulation bookkeeping
            w_first = {}
            w_last = {}
            for ci, (w, q, c0, nt, gt) in enumerate(calls):
                w_first.setdefault(w, ci)
                w_last[w] = ci

            psum_agg = None
            ti_in_w = 0
            tiles_w = 0
            for ci, (w, q, c0, nt, gt) in enumerate(calls):
                if w_first[w] == ci:
                    psum_agg = pagg.tile([128, WIN], f32, tag="pagg")
                    ti_in_w = 0
                    tiles_w = int(T[w].sum())
                slab = slpool.tile([128, MAX_CALL_TILES, DIM], bf16,
                                  tag=f"slab{ci % N_SLABS}")
                n_idx = nt * TILE
                creg = cregs[ci % N_SLABS]
                nc.gpsimd.reg_load(creg, cnts[0:1, ci:ci + 1])
                nc.gpsimd.dma_gather(
                    slab[:, :nt, :], z_ds[q][:],
                    idxs[:, (gt * TILE) // 16:(gt * TILE + n_idx) // 16],
                    n_idx, creg, DIM)
                # one batched one-hot build for the whole call
                S = spool.tile([TILE, MAX_CALL_TILES, WIN], bf16, tag="S")
                nc.vector.tensor_tensor(
                    out=S[:, :nt, :],
                    in0=iota[:].rearrange("p (t j) -> p t j", t=1)
                        .to_broadcast([TILE, nt, WIN]),
                    in1=dstv[:, gt:gt + nt]
                        .rearrange("p (t j) -> p t j", j=1)
                        .to_broadcast([TILE, nt, WIN]),
                    op=mybir.AluOpType.is_equal)
                for t in range(nt):
                    nc.tensor.matmul(
                        out=psum_agg[:], lhsT=slab[:, t, :], rhs=S[:, t, :],
                        start=(ti_in_w == 0), stop=(ti_in_w == tiles_w - 1))
                    ti_in_w += 1
                if w_last[w] == ci:
                    wlen = min(WIN, NPC - w * WIN)
                    aggT = apool.tile([128, WIN], f32, tag="aggT")
                    nc.vector.tensor_copy(out=aggT[:], in_=psum_agg[:])
                    psum_o = pout.tile([WIN, DIM], f32)
                    nc.tensor.matmul(out=psum_o[:], lhsT=aggT[:], rhs=Wt[:],
                                     start=True, stop=True)
                    osb = apool.tile([WIN, DIM], f32, tag="osb")
                    nc.vector.tensor_scalar(
                        out=osb[:], in0=psum_o[:],
                        scalar1=disw[:, w:w + 1], scalar2=None,
                        op0=mybir.AluOpType.mult)
                    nc.sync.dma_start(out=out_d[w * WIN:w * WIN + wlen, :],
                                      in_=osb[:wlen, :])
    nc.compile()
    return nc


def kernel(x, edge_index, W):
    _setup_concourse()
    from concourse.bass_utils import run_bass_kernel_spmd

    z, W32, T, calls, idx_arrs, dst_arrs, cnt_arrs, disw_arrs, iota = \
        _preprocess(x, edge_index, W)
    nc = _build(T, calls)

    in_maps = []
    for c in range(N_CORES):
        im = {"idxs": idx_arrs[c], "dstv": dst_arrs[c], "cnts": cnt_arrs[c],
              "disw": disw_arrs[c], "iota": iota, "W": W32}
        for q in range(NQ):
            im[f"z{q}"] = np.ascontiguousarray(z[q * CHUNK:(q + 1) * CHUNK])
        in_maps.append(im)
    res = run_bass_kernel_spmd(nc, in_maps, core_ids=list(range(N_CORES)))
    out = np.empty((N_NODES, DIM), np.float32)
    for c in range(N_CORES):
        out[c * NPC:(c + 1) * NPC] = res.results[c]["out"]
    return out


# revision 10
# speedup vs baseline: 1.1611x; 1.1604x over previous
"""GCNConv (PyG, bias=False) on 8 Trainium2 NeuronCores.

out = D^{-1/2} (A+I) D^{-1/2} (x @ W)

Strategy: the op is linear, so aggregate first, project second:
  z = dis * x                     (host; dis = rsqrt(degree), z stored bf16)
  aggT[f,d] = sum_{src->d} z[src] (device: dma_gather + one-hot matmul)
  out[d] = dis[d] * (agg[d] @ W)  (device: dis applied at the PSUM flush)

Node rows (outputs) are partitioned across the 8 cores; each core's edges are
sorted by (128-dst window, 25000-row src chunk, src).  Per (window, chunk)
group, dma_gather pulls z rows by local int16 source index in calls of up to
1024 rows; trailing padding uses idx -1 (descriptors skipped — per-call true
counts are value_load-ed from an input tensor so the shared SPMD NEFF works
for every core's edge counts).  One batched DVE tensor_tensor builds the
one-hot blocks S[e, t, d] = (dstoff[e,t] == d) for a whole call; the PE
accumulates aggT += slab_tile^T @ S_tile in PSUM over the window, then
aggT feeds matmul(lhsT=aggT, rhs=W) directly (contraction dim = feat is
already on partitions) and the flush multiplies by dis[dst] per partition.
No transposes are needed anywhere.  Gather slabs rotate through 6 fixed
slots, memset once, so rows skipped by short gathers always hold finite
stale data that the zero one-hot rows annihilate.
"""
import os
import sys

sys.path.insert(0, '/opt/trn_rl_repo')

import numpy as np

N_NODES = 100000
N_EDGES = 1600000
DIM = 128
N_CORES = 8
NPC = N_NODES // N_CORES          # dst rows per core (12500)
WIN = 128                         # dsts per window
NW = (NPC + WIN - 1) // WIN       # windows per core (98; last window 84 dsts)
CHUNK = 25000                     # src rows per gather-table chunk (int16 limit)
NQ = (N_NODES + CHUNK - 1) // CHUNK
TILE = 128                        # edges per tile
MAX_CALL_TILES = 8                # 1024 idxs per dma_gather (64-desc/engine cap)
N_SLABS = 6

_patched = False


def _setup_concourse():
    global _patched
    if _patched:
        return
    _patched = True
    import bass_rust
    import concourse.bass as bass
    import concourse.tile as tile

    # Walrus in this container allows exactly ONE sync-wait per instruction.
    # (1) Tile's end-of-context drain can carry several: split extra waits
    # onto chained Drain instructions.
    def _patched_drain_and_barrier(self, tick_clock, wait_clock):
        from concourse.vector_clock import ScopedClock
        nc = self.nc
        drain_inst = nc.sync.drain()
        wait_clock.add_sem_waits(drain_inst.ins,
                                 ScopedClock({None: tick_clock.global_clock}))
        si = drain_inst.ins.sync_info
        waits = list(si.on_wait or []) if si is not None else []
        if len(waits) > 1:
            si.on_wait = waits[:1]
            for w in waits[1:]:
                d2 = nc.sync.drain()
                d2.ins.sync_info = bass_rust.SyncInfo(on_wait=[w], on_update=[])
        nc.all_engine_barrier()
        popped = nc._tile_sem_poison_stack.pop()
        assert popped is self._sem_poison
        nc.clear_and_free_semaphores(list(self.sems.allocated().values()))
        nc.all_engine_barrier()

    tile.TileContext._drain_and_barrier = _patched_drain_and_barrier

    # (2) Any other instruction with >1 waits: move extras onto NoOp
    # carriers on the same engine immediately before it.
    def _legalize_waits(m):
        for f in m.functions:
            for blk in f.blocks:
                insts = blk.instructions
                out = []
                changed = False
                for inst in insts:
                    si = inst.sync_info
                    waits = list(si.on_wait or []) if si is not None else []
                    if len(waits) > 1:
                        changed = True
                        for k, w in enumerate(waits[:-1]):
                            nop = bass_rust.InstNoOp(
                                name=f"{inst.name}-wsplit{k}", ins=[], outs=[])
                            nop.engine = inst.engine
                            nop.sync_info = bass_rust.SyncInfo(
                                on_wait=[w], on_update=[])
                            out.append(nop)
                        si.on_wait = waits[-1:]
                    out.append(inst)
                if changed:
                    blk.instructions = out

    orig_to_json_bytes = bass.Bass.to_json_bytes
    if not getattr(bass.Bass, "_wsplit_patch", False):
        def _patched_to_json_bytes(self):
            _legalize_waits(self.m)
            return orig_to_json_bytes(self)
        bass.Bass.to_json_bytes = _patched_to_json_bytes
        bass.Bass._wsplit_patch = True


def _plan_calls(T, cnt):
    """Split each (w, q) group's tiles into dma_gather calls of <=8 tiles.
    Each call's num_idxs covers the max real count over cores (ceil to 16),
    so descriptor generation never touches pure-padding lanes.
    Returns [(w, q, c0, n_tiles, global_tile_idx, num_idxs)] in order."""
    calls = []
    gt = 0
    cmax = cnt.max(axis=0)                               # [NW, NQ]
    for w in range(NW):
        for q in range(NQ):
            tq = int(T[w, q])
            c0 = 0
            while c0 < tq:
                nt = min(MAX_CALL_TILES, tq - c0)
                real = int(cmax[w, q]) - c0 * TILE
                real = max(1, min(real, nt * TILE))
                n_idx = -(-real // 16) * 16
                calls.append((w, q, c0, nt, gt, n_idx))
                gt += nt
                c0 += nt
    return calls


def _preprocess(x, edge_index, W):
    """Host-side sharding: per-core padded edge arrays + shared schedule."""
    import ml_dtypes
    x = np.asarray(x, dtype=np.float32)
    W = np.asarray(W, dtype=np.float32)
    ei = np.asarray(edge_index)
    src = np.concatenate([ei[0], np.arange(N_NODES, dtype=ei.dtype)]).astype(np.int64)
    dst = np.concatenate([ei[1], np.arange(N_NODES, dtype=ei.dtype)]).astype(np.int64)

    deg = np.bincount(dst, minlength=N_NODES).astype(np.float32)
    dis = 1.0 / np.sqrt(np.maximum(deg, 1.0))
    z = (x * dis[:, None]).astype(ml_dtypes.bfloat16)    # gather table (bf16)

    core = dst // NPC
    dloc = dst - core * NPC
    w = dloc // WIN
    dstoff = (dloc - w * WIN).astype(np.float32)
    q = src // CHUNK
    srcloc = (src - q * CHUNK).astype(np.int16)

    key = (core * NW + w) * NQ + q
    order = np.lexsort((src, key))                       # by group, then src
    key_s = key[order]
    cnt = np.bincount(key, minlength=N_CORES * NW * NQ).reshape(N_CORES, NW, NQ)
    T = (-(-cnt // TILE)).max(axis=0)                    # [NW, NQ] tiles/group
    group_off = np.concatenate([[0], np.cumsum(T.reshape(-1) * TILE)])
    L = int(group_off[-1])                               # padded edges per core

    first_idx = np.searchsorted(key_s, np.arange(N_CORES * NW * NQ), side='left')
    rank = np.arange(key_s.size) - first_idx[key_s]
    pos = group_off[key_s % (NW * NQ)] + rank

    calls = _plan_calls(T, cnt)

    srcloc_s = srcloc[order]
    dstoff_s = dstoff[order]
    core_s = key_s // (NW * NQ)
    idx_arrs, dst_arrs, disw_arrs = [], [], []
    for c in range(N_CORES):
        m = core_s == c
        ia = np.zeros(L, np.int16)          # pad: gathers row 0 (valid, inert)
        da = np.full(L, -1.0, np.float32)   # pad: matches no iota column
        p = pos[m]
        ia[p] = srcloc_s[m]
        da[p] = dstoff_s[m]
        idx_arrs.append(np.ascontiguousarray(
            np.tile(ia.reshape(-1, 16).T, (8, 1))))
        dst_arrs.append(np.ascontiguousarray(
            da.reshape(-1, TILE).T.astype(ml_dtypes.bfloat16)))
        # dis of this core's dst rows, [128, NW] (partition p, window w)
        dw = np.zeros((128, NW), np.float32)
        rows = np.arange(NPC)
        dw[rows % WIN, rows // WIN] = dis[c * NPC + rows]
        disw_arrs.append(np.ascontiguousarray(dw))

    iota = np.ascontiguousarray(
        np.tile(np.arange(WIN, dtype=np.float32), (TILE, 1))
    ).astype(ml_dtypes.bfloat16)
    return z, W, T, calls, idx_arrs, dst_arrs, disw_arrs, iota


def _build(T, calls):
    """Build the shared SPMD bass program from the schedule."""
    import concourse.bacc as bacc
    import concourse.mybir as mybir
    import concourse.tile as tile

    tot_tiles = int(T.sum())
    L = tot_tiles * TILE
    bf16 = mybir.dt.bfloat16
    f32 = mybir.dt.float32

    nc = bacc.Bacc("TRN2", target_bir_lowering=False, debug=False)
    z_ds = [nc.dram_tensor(f"z{q}", [min(CHUNK, N_NODES - q * CHUNK), DIM],
                           bf16, kind="ExternalInput")
            for q in range(NQ)]
    idx_d = nc.dram_tensor("idxs", [128, L // 16], mybir.dt.int16, kind="ExternalInput")
    dst_d = nc.dram_tensor("dstv", [128, tot_tiles], bf16, kind="ExternalInput")
    disw_d = nc.dram_tensor("disw", [128, NW], f32, kind="ExternalInput")
    iota_d = nc.dram_tensor("iota", [128, WIN], bf16, kind="ExternalInput")
    W_d = nc.dram_tensor("W", [DIM, DIM], f32, kind="ExternalInput")
    out_d = nc.dram_tensor("out", [NPC, DIM], f32, kind="ExternalOutput")

    with tile.TileContext(nc) as tc:
        with tc.tile_pool(name="const", bufs=1) as cpool, \
             tc.tile_pool(name="slabs", bufs=1) as slpool, \
             tc.tile_pool(name="sel", bufs=4) as spool, \
             tc.tile_pool(name="stage", bufs=3) as apool, \
             tc.tile_pool(name="pagg", bufs=3, space="PSUM") as pagg, \
             tc.tile_pool(name="pout", bufs=2, space="PSUM") as pout:

            idxs = cpool.tile([128, L // 16], mybir.dt.int16)
            nc.sync.dma_start(out=idxs[:], in_=idx_d[:])
            dstv = cpool.tile([128, tot_tiles], bf16)
            nc.sync.dma_start(out=dstv[:], in_=dst_d[:])
            disw = cpool.tile([128, NW], f32)
            nc.sync.dma_start(out=disw[:], in_=disw_d[:])
            iota = cpool.tile([128, WIN], bf16)
            nc.sync.dma_start(out=iota[:], in_=iota_d[:])
            Wt = cpool.tile([DIM, DIM], f32)
            nc.sync.dma_start(out=Wt[:], in_=W_d[:])

            # fixed gather slots, memset once -> unwritten rows stay finite
            slabs = []
            for i in range(N_SLABS):
                s = slpool.tile([128, MAX_CALL_TILES, DIM], bf16, tag=f"slab{i}")
                nc.vector.memset(s[:], 0.0)
                slabs.append(s)

            # group calls by window for PSUM accumulation bookkeeping
            w_first = {}
            w_last = {}
            for ci, (w, q, c0, nt, gt, n_idx) in enumerate(calls):
                w_first.setdefault(w, ci)
                w_last[w] = ci

            psum_agg = None
            ti_in_w = 0
            tiles_w = 0
            for ci, (w, q, c0, nt, gt, n_idx) in enumerate(calls):
                if w_first[w] == ci:
                    psum_agg = pagg.tile([128, WIN], f32, tag="pagg")
                    ti_in_w = 0
                    tiles_w = int(T[w].sum())
                slab = slpool.tile([128, MAX_CALL_TILES, DIM], bf16,
                                  tag=f"slab{ci % N_SLABS}")
                ntg = -(-n_idx // TILE)     # tiles actually gathered
                nc.gpsimd.dma_gather(
                    slab[:, :ntg, :], z_ds[q][:],
                    idxs[:, (gt * TILE) // 16:(gt * TILE + n_idx + 15) // 16],
                    n_idx, n_idx, DIM)
                # one batched one-hot build for the whole call
                S = spool.tile([TILE, MAX_CALL_TILES, WIN], bf16, tag="S")
                nc.vector.tensor_tensor(
                    out=S[:, :nt, :],
                    in0=iota[:].rearrange("p (t j) -> p t j", t=1)
                        .to_broadcast([TILE, nt, WIN]),
                    in1=dstv[:, gt:gt + nt]
                        .rearrange("p (t j) -> p t j", j=1)
                        .to_broadcast([TILE, nt, WIN]),
                    op=mybir.AluOpType.is_equal)
                for t in range(nt):
                    nc.tensor.matmul(
                        out=psum_agg[:], lhsT=slab[:, t, :], rhs=S[:, t, :],
                        start=(ti_in_w == 0), stop=(ti_in_w == tiles_w - 1))
                    ti_in_w += 1
                if w_last[w] == ci:
                    wlen = min(WIN, NPC - w * WIN)
                    aggT = apool.tile([128, WIN], f32, tag="aggT")
                    nc.vector.tensor_copy(out=aggT[:], in_=psum_agg[:])
                    psum_o = pout.tile([WIN, DIM], f32)
                    nc.tensor.matmul(out=psum_o[:], lhsT=aggT[:], rhs=Wt[:],
                                     start=True, stop=True)
                    osb = apool.tile([WIN, DIM], f32, tag="osb")
                    nc.vector.tensor_scalar(
                        out=osb[:], in0=psum_o[:],
                        scalar1=disw[:, w:w + 1], scalar2=None,
                        op0=mybir.AluOpType.mult)
                    nc.sync.dma_start(out=out_d[w * WIN:w * WIN + wlen, :],
                                      in_=osb[:wlen, :])
    nc.compile()
    return nc


def kernel(x, edge_index, W):
    _setup_concourse()
    from concourse.bass_utils import run_bass_kernel_spmd

    z, W32, T, calls, idx_arrs, dst_arrs, disw_arrs, iota = \
        _preprocess(x, edge_index, W)
    nc = _build(T, calls)

    in_maps = []
    for c in range(N_CORES):
        im = {"idxs": idx_arrs[c], "dstv": dst_arrs[c],
              "disw": disw_arrs[c], "iota": iota, "W": W32}
        for q in range(NQ):
            im[f"z{q}"] = np.ascontiguousarray(z[q * CHUNK:(q + 1) * CHUNK])
        in_maps.append(im)
    res = run_bass_kernel_spmd(nc, in_maps, core_ids=list(range(N_CORES)))
    out = np.empty((N_NODES, DIM), np.float32)
    for c in range(N_CORES):
        out[c * NPC:(c + 1) * NPC] = res.results[c]["out"]
    return out
